# Initial kernel scaffold
#
"""Trainium2 Bass kernel for nn_EnhancedSyntheticEmbeddingLayer.

Strategy: pure data parallelism — B=8 batch elements, one per NeuronCore.
Forward-only, so no collectives. Each core runs the full hybrid block on its
(S=1024, D=512) sequence.

Layouts: T-layout [128 part, DO=4 d-tiles, S free] (features on partitions),
N-layout [128 part, SO=8 t-tiles, D free] (tokens on partitions). All matmuls
bf16 (fp32 PE is 4x slower), fp32 PSUM accumulation.

v2 changes vs baseline (1293us -> ~1090us):
- BitNet ternary quantization + liquid gates precomputed on HOST (weights-only
  computation); the TWQ streaming pass and in-kernel quantize are gone.
- Attention: single logits pass per head in q-layout (partitions=q, free=k);
  the per-q max becomes a per-partition ACT bias and the exp emits its own
  row-sum via accum_out (no DRAM roundtrips, no second logits matmul);
  normalization folded into wq before the PE transpose to k-layout.
- Mamba: scans consolidated to NG=4 n-channels per instruction via chained
  multi-row scans with a zeroed-first-decay-column reset (memset emitted
  BEFORE the dA exps on disjoint slices so the in-order DVE queue never
  stalls); hC computed in place on hsc; hsc double-buffered so the y-reduce
  matmuls never block the next group's scan. Mamba phases run at the DVE
  floor (~13.3us per 4-channel group).
- bnb/bo/fcb biases applied on DVE instead of 1-row PE matmuls; act_quant
  runs in N-layout (per-partition scales, no DRAM roundtrip).

Known dead ends (measured): gpsimd tensor_tensor is 3.5x slower than DVE and
its scan/free-axis-reduce opcodes don't exist on Pool; dma_start_transpose
races its consumers under load (multi-core corruption) and contends for
shared DMA bandwidth; matmul free size is capped at 512 (one PSUM bank) by
the ISA even though bass compiles 1024.
"""
import numpy as np
import ml_dtypes

import concourse.bass as bass
import concourse.mybir as mybir
import concourse.tile as tile
from concourse import bacc
from concourse.bass_utils import run_bass_kernel_spmd
from contextlib import ExitStack

F32 = mybir.dt.float32
F16 = mybir.dt.float16
AF = mybir.ActivationFunctionType
OP = mybir.AluOpType
AX = mybir.AxisListType

P = 128
S = 1024
D = 512
N = 16
L = 3
SO = 8
DO = 4
NH = 4
NG = 4   # n's per scan group
MAGIC = 1.5 * 2.0**23

DBG = False

MAMBAS = ("m1", "sm", "m2")
H512 = [(0, 512), (512, 1024)]


def _decl(nc, use_mask):
    t = {}
    def inp(name, shape, dt):
        t[name] = nc.declare_dram_parameter(name, list(shape), dt, isOutput=False)
    def outp(name, shape, dt):
        t[name] = nc.declare_dram_parameter(name, list(shape), dt, isOutput=True)

    inp("xT", (P, DO, S), F16)
    inp("xN", (P, SO, D), F32)
    inp("mask_bf", (1, S), F16)
    inp("mask_N", (P, SO), F32)
    if use_mask:
        inp("maskbias_k", (1, S), F32)
    for m in MAMBAS:
        inp(f"{m}_Wdt", (P, DO, D), F16)
        inp(f"{m}_prm", (P, 72), F32)        # [0:64]=A_sc [64:68]=bdt [68:72]=D
        inp(f"{m}_wbc", (P, 2, DO, N), F16)  # [:,0]=WB [:,1]=WC
    for w in ("Wq", "Wk", "Wv", "Wo", "Wg", "Wm", "fcW"):
        inp(w, (P, DO, D), F16)
    inp("bo_bc", (P, D), F16)
    inp("fcb", (1, D), F16)
    inp("TW", (L, P, SO, S), F16)            # host-quantized ternary weights
    inp("bnb", (L, 1, S), F16)
    inp("lqW", (L, P, DO, D), F16)
    inp("lqb", (P, DO, L), F32)
    inp("gpq", (P, 2, DO, L), F32)           # host: [g/L, (1-g)/L]
    inp("lnsc", (1, D), F32)
    inp("lnbi", (1, D), F32)
    inp("cpack", (P, 2, P), F16)             # [:,0]=I128 [:,1]=ones128
    outp("out", (P, SO, D), F32)

    t["stg_b"] = nc.dram_tensor("stg_b", [1, N * S], F16)
    t["stg_c"] = nc.dram_tensor("stg_c", [1, N * S], F16)
    t["stg_b2"] = nc.dram_tensor("stg_b2", [1, N * S], F16)
    t["stg_c2"] = nc.dram_tensor("stg_c2", [1, N * S], F16)
    t["stg_s"] = nc.dram_tensor("stg_s", [NH, S], F32)
    t["stg_s2"] = nc.dram_tensor("stg_s2", [NH, S], F32)
    t["stg_am"] = nc.dram_tensor("stg_am", [2, S], F32)
    if DBG:
        outp("d_delta1", (P, DO, S), F16)
        outp("d_u1", (P, DO, S), F16)
        outp("d_h1", (P, DO, S), F16)
        outp("d_h2", (P, SO, D), F16)
        outp("d_qT", (P, DO, S), F16)
        outp("d_kT", (P, DO, S), F16)
        outp("d_vN", (P, SO, D), F16)
        outp("d_h3", (P, DO, S), F16)
        outp("d_h4", (P, DO, S), F16)
        outp("d_h5", (P, DO, S), F16)
        outp("d_h5q", (P, DO, S), F16)
        outp("d_h6", (P, DO, S), F16)
        outp("d_preln", (P, SO, D), F32)
    return t


def build_mamba(nc, tc, t, cst, spool, wring, psum2, psum1,
                pfx, x_T, out_h, sparse, dbg=None,
                pre_bc=None, skip_bc=False):
    """x_T bf16 [P,DO,S] -> out_h bf16 [P,DO,S]. Opens its own phase pools."""
    v, sc, sy = nc.vector, nc.scalar, nc.sync
    with nc.named_scope(f"mamba_{pfx}"), \
         tc.tile_pool(name=f"{pfx}_p1", bufs=1) as mp1, \
         tc.tile_pool(name=f"{pfx}_pa", bufs=2) as mpa, \
         tc.tile_pool(name=f"{pfx}_ph", bufs=2) as mph, \
         tc.tile_pool(name=f"{pfx}_ps", bufs=1) as mps:
        Wdt = wring.tile([P, DO, D], F16, tag="w4", name="Wdt")
        sy.dma_start(Wdt[:], t[f"{pfx}_Wdt"][:])
        prm = mp1.tile([P, 72], F32, tag="prm", name="prm")
        sy.dma_start(prm[:], t[f"{pfx}_prm"][:])
        wbc = mp1.tile([P, 2, DO, N], F16, tag="wbc", name="wbc")
        sy.dma_start(wbc[:], t[f"{pfx}_wbc"][:])
        A_sc = prm[:, 0:64]
        bdt = prm[:, 64:68]
        D_sc = prm[:, 68:72]

        # Bm^T, Cm^T -> DRAM -> broadcast to all partitions
        stg_bc = (t["stg_b2"], t["stg_c2"]) if skip_bc else \
                 (t["stg_b"], t["stg_c"])
        if not skip_bc:
            for ci, stg in enumerate(stg_bc):
                ps = psum2.tile([P, S], F32, tag="p1024", name="ps_bc")
                for lo, hi in H512:
                    for ko in range(DO):
                        nc.tensor.matmul(ps[:N, lo:hi], wbc[:, ci, ko, :],
                                         x_T[:, ko, lo:hi],
                                         start=(ko == 0), stop=(ko == DO - 1))
                bc_sb = spool.tile([N, S], F16, tag="s4", name="bc_sb")
                v.tensor_copy(bc_sb[:], ps[:N, :])
                sy.dma_start(stg.rearrange("o (n t) -> (o n) t", n=N), bc_sb[:])
        # broadcast reads split per scan group so the first group's dBx can
        # start as soon as its slice lands (shorter phase-entry latency)
        BmF = mp1.tile([P, N, S], F16, tag="BmF", name="BmF")
        CmF = mp1.tile([P, N, S], F16, tag="CmF", name="CmF")
        for g in range(N // NG):
            sl = slice(g * NG * S, (g + 1) * NG * S)
            sy.dma_start(
                BmF[:, g * NG:(g + 1) * NG, :].rearrange("p n t -> p (n t)"),
                stg_bc[0][0:1, sl].to_broadcast((P, NG * S)))
            sy.dma_start(
                CmF[:, g * NG:(g + 1) * NG, :].rearrange("p n t -> p (n t)"),
                stg_bc[1][0:1, sl].to_broadcast((P, NG * S)))

        # delta^T = softplus(Wdt^T @ x^T + bdt)  (bf16)
        delta = mp1.tile([P, DO, S], F16, tag="delta", name="delta")
        for mo in range(DO):
            ps = psum2.tile([P, S], F32, tag="p1024", name="ps_d")
            for lo, hi in H512:
                for ko in range(DO):
                    nc.tensor.matmul(ps[:, lo:hi],
                                     Wdt[:, ko, mo * P:(mo + 1) * P],
                                     x_T[:, ko, lo:hi],
                                     start=(ko == 0), stop=(ko == DO - 1))
            # softplus(x) = ln(1 + exp(x)); no softplus LUT in this build.
            # |x| stays O(10) here so exp can't overflow fp32.
            e1 = spool.tile([P, S], F32, tag="s4", name="e1")
            sc.activation(e1[:], ps[:], AF.Exp, bias=bdt[:, mo:mo + 1])
            sc.activation(delta[:, mo, :], e1[:], AF.Ln, bias=1.0)

        if sparse:
            # delta = where(delta > mean_d(delta), delta, 0); mean over d per t
            psm = psum2.tile([P, S], F32, tag="p1024", name="psm")
            for lo, hi in H512:
                for ko in range(DO):
                    nc.tensor.matmul(psm[:, lo:hi], cst["ones128"],
                                     delta[:, ko, lo:hi],
                                     start=(ko == 0), stop=(ko == DO - 1))
            for mo in range(DO):
                msk = spool.tile([P, S], F16, tag="s4", name="msk")
                v.scalar_tensor_tensor(msk[:], psm[:], 1.0 / D, delta[:, mo, :],
                                       OP.mult, OP.is_lt)
                v.tensor_tensor(delta[:, mo, :], delta[:, mo, :], msk[:], OP.mult)

        # u = delta * x (per d-block, so dk 0's scan chain starts as soon as
        # delta[:, 0, :] lands rather than after the whole projection)
        u = mp1.tile([P, DO, S], F16, tag="u", name="u")
        for mo in range(DO):
            v.tensor_tensor(u[:, mo, :], delta[:, mo, :], x_T[:, mo, :],
                            OP.mult)

        # scans: per dk, 2 groups of NG=8 n's. Within a group one chained
        # multi-row scan handles all 8 recurrences; dA[:,j,0]=0 for j>0 resets
        # the carried state at each row boundary (the true a_0 only multiplies
        # h_{-1}=0, so losing it is harmless).
        for dk in range(DO):
            ps_y = psum1.tile([P, S], F32, tag="ypsum", name="ps_y")
            first = True
            for g in range(N // NG):
                dA = mpa.tile([P, NG, S], F16, tag="dA", name="dA")
                # zero first decay col of rows 1..NG-1 (scan chain reset);
                # emitted BEFORE the exps (disjoint slices) so the in-order
                # DVE queue never stalls on the ACT exps here.
                v.memset(dA[:, 1:NG, 0:1], 0.0)
                for j in range(NG):
                    n = g * NG + j
                    if j == 0:
                        sc.activation(dA[:, 0, :], delta[:, dk, :], AF.Exp,
                                      scale=A_sc[:, dk * N + n: dk * N + n + 1])
                    else:
                        sc.activation(dA[:, j, 1:], delta[:, dk, 1:], AF.Exp,
                                      scale=A_sc[:, dk * N + n: dk * N + n + 1])
                dBx = mps.tile([P, NG, S], F16, tag="dBx", name="dBx")
                v.tensor_tensor(dBx[:],
                                u[:, dk, None, :].to_broadcast((P, NG, S)),
                                BmF[:, g * NG:(g + 1) * NG, :], OP.mult)
                hsc = mph.tile([P, NG, S], F16, tag="hsc", name="hsc")
                v.tensor_tensor_scan(hsc.rearrange("p g s -> p (g s)"),
                                     dA.rearrange("p g s -> p (g s)"),
                                     dBx.rearrange("p g s -> p (g s)"),
                                     0.0, OP.mult, OP.add)
                # hC in place on hsc (elementwise same-index aliasing is safe)
                v.tensor_tensor(hsc[:], hsc[:], CmF[:, g * NG:(g + 1) * NG, :],
                                OP.mult)
                for j in range(NG):
                    last = (g == N // NG - 1) and (j == NG - 1)
                    for lo, hi in H512:
                        nc.tensor.matmul(ps_y[:, lo:hi], cst["I128"],
                                         hsc[:, j, lo:hi], start=first, stop=last)
                    first = False
            v.scalar_tensor_tensor(out_h[:, dk, :], x_T[:, dk, :],
                                   D_sc[:, dk:dk + 1], ps_y[:], OP.mult, OP.add)
            if pre_bc is not None:
                pre_bc(dk)
        if dbg == 1 and DBG:
            sy.dma_start(t["d_delta1"][:], delta[:])
            sy.dma_start(t["d_u1"][:], u[:])


def build_attention(nc, tc, t, cst, spool, wring, psum2, psum1, h1, h2,
                    use_mask):
    """h1 bf16 [P,DO,S] -> h2 bf16 [P,SO,D].

    Single logits pass per head in q-layout (partitions=q, free=k): the per-q
    max is a per-partition ACT bias. w is then PE-transposed to k-layout for
    the sum/o matmuls. Softmax normalization is deferred past the output
    projection (a per-token ACT scale in N-layout) since diag(r)(o^T Wo) =
    (diag(r) o^T) Wo."""
    v, sc, sy = nc.vector, nc.scalar, nc.sync
    with nc.named_scope("attn"), tc.tile_pool(name="attnp", bufs=1) as ap, \
         tc.tile_pool(name="attnw", bufs=2) as aw:
        Wq = wring.tile([P, DO, D], F16, tag="w4", name="Wq")
        sc.dma_start(Wq[:], t["Wq"][:])
        Wk = wring.tile([P, DO, D], F16, tag="w4", name="Wk")
        sc.dma_start(Wk[:], t["Wk"][:])
        Wv = wring.tile([P, DO, D], F16, tag="w4", name="Wv")
        sc.dma_start(Wv[:], t["Wv"][:])

        qT = ap.tile([P, DO, S], F16, tag="qT", name="qT")
        kT = ap.tile([P, DO, S], F16, tag="kT", name="kT")
        for (W, dst) in ((Wq, qT), (Wk, kT)):
            for mo in range(DO):
                ps = psum2.tile([P, S], F32, tag="p1024", name="ps_qk")
                for lo, hi in H512:
                    for ko in range(DO):
                        nc.tensor.matmul(ps[:, lo:hi],
                                         W[:, ko, mo * P:(mo + 1) * P],
                                         h1[:, ko, lo:hi],
                                         start=(ko == 0), stop=(ko == DO - 1))
                sc.copy(dst[:, mo, :], ps[:])
        vN = ap.tile([P, SO, D], F16, tag="vN", name="vN")
        for st in range(SO):
            ps = psum2.tile([P, D], F32, tag="p512", name="ps_v")
            for ko in range(DO):
                nc.tensor.matmul(ps[:], h1[:, ko, st * P:(st + 1) * P],
                                 Wv[:, ko, :], start=(ko == 0), stop=(ko == DO - 1))
            sc.copy(vN[:, st, :], ps[:])

        if use_mask:
            mrow = spool.tile([1, S], F32, tag="s4", name="mrow")
            sy.dma_start(mrow[:], t["maskbias_k"][:])

        scale = 1.0 / float(np.sqrt(D // NH))
        oT = ap.tile([P, DO, S], F16, tag="oT", name="oT")
        for hd in range(NH):
            # logits in q-layout; per-q max -> ACT bias for the exp; row sums
            # as q-layout free reduces (no matmul, no DRAM roundtrips);
            # normalization folded into wq before the transpose.
            wq = aw.tile([P, SO, S], F16, tag="wq", name="wq")
            mxn = spool.tile([P, SO], F32, tag="s4", name="mxn")
            sums = spool.tile([P, SO], F32, tag="s4", name="sums")
            for qt in range(SO):
                # rotate logits through 3 PSUM slots (2x p1024 + ypsum) so
                # the max/exp latency pipeline doesn't stall the PE
                if qt % 3 == 2:
                    ps_l = psum1.tile([P, S], F32, tag="ypsum", name="ps_l")
                else:
                    ps_l = psum2.tile([P, S], F32, tag="p1024", name="ps_l")
                for lo, hi in H512:
                    nc.tensor.matmul(ps_l[:, lo:hi],
                                     qT[:, hd, qt * P:(qt + 1) * P],
                                     kT[:, hd, lo:hi], start=True,
                                     stop=(not use_mask))
                    if use_mask:
                        # masked keys: +(-1e9) -> exp()=0 (scaled by `scale`,
                        # still ~-1e8, far below fp32 exp underflow)
                        nc.tensor.matmul(ps_l[:, lo:hi], cst["ones1"],
                                         mrow[0:1, lo:hi], start=False,
                                         stop=True)
                mx = spool.tile([P, 1], F32, tag="s4", name="mx")
                v.tensor_reduce(mx[:], ps_l[:], AX.X, OP.max)
                v.tensor_scalar(mxn[:, qt:qt + 1], mx[:], -scale, 0.0,
                                OP.mult, OP.add)
                # the exp emits its own row sum via the ACT accumulator
                sc.activation(wq[:, qt, :], ps_l[:], AF.Exp, scale=scale,
                              bias=mxn[:, qt:qt + 1],
                              accum_out=sums[:, qt:qt + 1])
            r_q = spool.tile([P, SO], F32, tag="s4", name="r_q")
            v.reciprocal(r_q[:], sums[:])
            for qt in range(SO):
                v.tensor_scalar_mul(wq[:, qt, :], wq[:, qt, :],
                                    r_q[:, qt:qt + 1])
            # PE-transpose normalized w to k-layout [k, q]
            wT = aw.tile([P, SO, S], F16, tag="wT", name="wT")
            for kt in range(SO):
                pst = psum2.tile([P, SO, P], F16, tag="p512", name="pst")
                for qt in range(SO):
                    nc.tensor.transpose(pst[:, qt, :],
                                        wq[:, qt, kt * P:(kt + 1) * P],
                                        cst["I128"])
                sc.copy(wT[:, kt, :], pst.rearrange("p a b -> p (a b)"))
            # o = w^T v per q-half (1-bank PSUM tiles)
            for qh, (lo, hi) in enumerate(H512):
                ps_o = psum2.tile([P, 512], F32, tag="p512", name="ps_o")
                for kt in range(SO):
                    nc.tensor.matmul(ps_o[:],
                                     vN[:, kt, hd * P:(hd + 1) * P],
                                     wT[:, kt, lo:hi],
                                     start=(kt == 0), stop=(kt == SO - 1))
                sc.copy(oT[:, hd, lo:hi], ps_o[:])

        if DBG:
            sy.dma_start(t["d_qT"][:], qT[:])
            sy.dma_start(t["d_kT"][:], kT[:])
            sy.dma_start(t["d_vN"][:], vN[:])
        Wo = wring.tile([P, DO, D], F16, tag="w4", name="Wo")
        sc.dma_start(Wo[:], t["Wo"][:])
        bo_bc = spool.tile([P, D], F16, tag="s4", name="bo_bc")
        sy.dma_start(bo_bc[:], t["bo_bc"][:])
        for st in range(SO):
            ps = psum2.tile([P, D], F32, tag="p512", name="ps_a")
            for ko in range(DO):
                nc.tensor.matmul(ps[:], oT[:, ko, st * P:(st + 1) * P],
                                 Wo[:, ko, :], start=(ko == 0),
                                 stop=(ko == DO - 1))
            v.tensor_tensor(h2[:, st, :], ps[:], bo_bc[:], OP.add)


def build_bitnet(nc, tc, t, cst, spool, wring, psum2, psum1, h2, h3):
    """L x (ternary seq-dense + liquid gate), mean over L.
    h2 bf16 [P,SO,D] -> h3 bf16 [P,DO,S]. TW host-precomputed."""
    v, sc, sy = nc.vector, nc.scalar, nc.sync
    with nc.named_scope("bitnet"), \
         tc.tile_pool(name="bitp1", bufs=1) as bp1, \
         tc.tile_pool(name="bitp2", bufs=2) as bp2:
        lqW = [wring.tile([P, DO, D], F16, tag="w4", name=f"lqW{l}")
               for l in range(L)]
        for l in range(L):
            sc.dma_start(lqW[l][:], t["lqW"][l])
        # bnb broadcast to all partitions; added on DVE (frees PE matmuls)
        bnb = bp1.tile([P, L, S], F16, tag="bnb", name="bnb")
        for l in range(L):
            sy.dma_start(bnb[:, l, :], t["bnb"][l].to_broadcast((P, S)))
        bprm = bp1.tile([P, DO, L], F32, tag="bprm", name="bprm")
        sy.dma_start(bprm[:], t["lqb"][:])
        gpq = bp1.tile([P, 2, DO, L], F32, tag="gpq", name="gpq")
        sy.dma_start(gpq[:], t["gpq"][:])

        h3f = bp1.tile([P, DO, S], F16, tag="h3f", name="h3f")
        bdT = bp1.tile([P, DO, S], F16, tag="bdT", name="bdT")
        for l in range(L):
            TW = bp2.tile([P, SO, S], F16, tag="TWs", name="TW")
            sc.dma_start(TW[:], t["TW"][l])
            for mo in range(DO):
                ps_bd = psum2.tile([P, S], F32, tag="p1024", name="ps_bd")
                for lo, hi in H512:
                    for ko in range(SO):
                        nc.tensor.matmul(ps_bd[:, lo:hi],
                                         h2[:, ko, mo * P:(mo + 1) * P],
                                         TW[:, ko, lo:hi],
                                         start=(ko == 0), stop=(ko == SO - 1))
                v.tensor_tensor(bdT[:, mo, :], ps_bd[:], bnb[:, l, :], OP.add)
            for mo in range(DO):
                ps_tg = psum1.tile([P, S], F32, tag="ypsum", name="ps_tg")
                for lo, hi in H512:
                    for ko in range(DO):
                        nc.tensor.matmul(ps_tg[:, lo:hi],
                                         lqW[l][:, ko, mo * P:(mo + 1) * P],
                                         bdT[:, ko, lo:hi],
                                         start=(ko == 0), stop=(ko == DO - 1))
                th = spool.tile([P, S], F16, tag="s4", name="th")
                sc.activation(th[:], ps_tg[:], AF.Tanh, bias=bprm[:, mo, l:l + 1])
                if l == 0:
                    v.tensor_scalar_mul(h3f[:, mo, :], bdT[:, mo, :],
                                        gpq[:, 0, mo, l:l + 1])
                else:
                    v.scalar_tensor_tensor(h3f[:, mo, :], bdT[:, mo, :],
                                           gpq[:, 0, mo, l:l + 1], h3f[:, mo, :],
                                           OP.mult, OP.add)
                v.scalar_tensor_tensor(h3f[:, mo, :], th[:],
                                       gpq[:, 1, mo, l:l + 1], h3f[:, mo, :],
                                       OP.mult, OP.add)
        v.tensor_copy(h3[:], h3f[:])


def build_program(use_mask=False):
    nc = bacc.Bacc("TRN2")
    t = _decl(nc, use_mask)
    with tile.TileContext(nc) as tc, ExitStack() as ctx:
        gpool = ctx.enter_context(tc.tile_pool(name="gpool", bufs=1))
        wring = ctx.enter_context(tc.tile_pool(name="wring", bufs=4))
        spool = ctx.enter_context(tc.tile_pool(name="spool", bufs=7))
        psum2 = ctx.enter_context(tc.tile_pool(name="psum2", bufs=2, space="PSUM"))
        psum1 = ctx.enter_context(tc.tile_pool(name="psum1", bufs=1, space="PSUM"))
        v, sc, sy = nc.vector, nc.scalar, nc.sync

        cpk = gpool.tile([P, 2, P], F16, tag="cpack", name="cpk")
        sy.dma_start(cpk[:], t["cpack"][:])
        cst = {"I128": cpk[:, 0, :], "ones128": cpk[:, 1, :],
               "ones1": cpk[0:1, 1, :], "cpk": cpk}
        res = gpool.tile([P, SO, D], F16, tag="res", name="res")
        xmT = gpool.tile([P, DO, S], F16, tag="hB", name="xmT")
        h2N = gpool.tile([P, SO, D], F16, tag="h2N", name="h2N")

        with tc.tile_pool(name="initp", bufs=1) as ip:
            if use_mask:
                xTr = ip.tile([P, DO, S], F16, tag="xTr", name="xTr")
                sy.dma_start(xTr[:], t["xT"][:])
                maskb = ip.tile([P, S], F16, tag="maskb", name="maskb")
                sy.dma_start(maskb[:], t["mask_bf"][0:1, :].to_broadcast((P, S)))
                v.tensor_tensor(xmT[:], xTr[:],
                                maskb[:, None, :].to_broadcast((P, DO, S)),
                                OP.mult)
            else:
                # mask is all-ones: x*mask == x; xT per d-block so m1's
                # ko=0 matmuls fire after 0.5MB instead of the full 2MB
                for j in range(DO):
                    sy.dma_start(xmT[:, j, :], t["xT"][:, j, :])

        h1 = gpool.tile([P, DO, S], F16, tag="hA", name="h1")
        build_mamba(nc, tc, t, cst, spool, wring, psum2, psum1,
                    "m1", xmT, h1, sparse=False, dbg=1)

        # residual (x + fc_b) in N layout. Emitted AFTER m1 so the 2MB fp32
        # xN read queues behind m1's critical weight DMAs on the sync queue
        # instead of head-blocking them; res isn't needed until the finale.
        with tc.tile_pool(name="resp", bufs=1) as rp:
            xNr = rp.tile([P, SO, D], F32, tag="xNr", name="xNr")
            sy.dma_start(xNr[:], t["xN"][:])
            fcb_bc = spool.tile([P, D], F16, tag="s4", name="fcb_bc")
            sy.dma_start(fcb_bc[:], t["fcb"][0:1, :].to_broadcast((P, D)))
            if use_mask:
                maskN = spool.tile([P, SO], F32, tag="s4", name="maskN")
                sy.dma_start(maskN[:], t["mask_N"][:])
                for st in range(SO):
                    v.tensor_scalar_mul(res[:, st, :], xNr[:, st, :],
                                        maskN[:, st:st + 1])
                for st in range(SO):
                    v.tensor_tensor(res[:, st, :], res[:, st, :], fcb_bc[:],
                                    OP.add)
            else:
                v.tensor_tensor(res[:], xNr[:],
                                fcb_bc[:, None, :].to_broadcast((P, SO, D)),
                                OP.add)
        if DBG:
            sy.dma_start(t["d_h1"][:], h1[:])

        build_attention(nc, tc, t, cst, spool, wring, psum2, psum1, h1, h2N,
                        use_mask)
        if DBG:
            sy.dma_start(t["d_h2"][:], h2N[:])

        h3 = gpool.tile([P, DO, S], F16, tag="hB", name="h3")
        build_bitnet(nc, tc, t, cst, spool, wring, psum2, psum1, h2N, h3)
        if DBG:
            sy.dma_start(t["d_h3"][:], h3[:])

        h4 = gpool.tile([P, DO, S], F16, tag="hA", name="h4")
        # m2's B/C projections are emitted inside sm's dk loop (engines run
        # their queues in order, so only emission-interleaved work can
        # execute during sm's scans). psum2 is idle during sm's scan phase,
        # so the two accumulators can hold both ring slots across the dks.
        wbc2 = gpool.tile([P, 2, DO, N], F16, tag="wbc2", name="wbc2")
        sy.dma_start(wbc2[:], t["m2_wbc"][:])
        bc2_state = {}

        def pre_bc_m2(dk):
            if dk == 0:
                bc2_state["pb"] = psum2.tile([P, S], F32, tag="p1024",
                                             name="ps_b2")
                bc2_state["pc"] = psum2.tile([P, S], F32, tag="p1024",
                                             name="ps_c2")
            for ci, key in ((0, "pb"), (1, "pc")):
                ps = bc2_state[key]
                for lo, hi in H512:
                    nc.tensor.matmul(ps[:N, lo:hi], wbc2[:, ci, dk, :],
                                     h4[:, dk, lo:hi],
                                     start=(dk == 0), stop=(dk == DO - 1))
            if dk == DO - 1:
                for key, stg in (("pb", t["stg_b2"]), ("pc", t["stg_c2"])):
                    bc_sb = spool.tile([N, S], F16, tag="s4", name="bc_sb2")
                    v.tensor_copy(bc_sb[:], bc2_state[key][:N, :])
                    sy.dma_start(stg.rearrange("o (n t) -> (o n) t", n=N),
                                 bc_sb[:])

        build_mamba(nc, tc, t, cst, spool, wring, psum2, psum1,
                    "sm", h3, h4, sparse=True, pre_bc=pre_bc_m2)
        if DBG:
            sy.dma_start(t["d_h4"][:], h4[:])
        h5 = gpool.tile([P, DO, S], F16, tag="hB", name="h5")
        build_mamba(nc, tc, t, cst, spool, wring, psum2, psum1,
                    "m2", h4, h5, sparse=False, skip_bc=True)
        if DBG:
            sy.dma_start(t["d_h5"][:], h5[:])

        with nc.named_scope("final"), tc.tile_pool(name="finp", bufs=1) as fp:
            lnp = fp.tile([P, 2, D], F32, tag="lnpack", name="lnp")
            sy.dma_start(lnp[:, 0, :], t["lnsc"][0:1, :].to_broadcast((P, D)))
            sy.dma_start(lnp[:, 1, :], t["lnbi"][0:1, :].to_broadcast((P, D)))
            # act_quant: PE-transpose to N layout for per-token absmax over d
            h5N = fp.tile([P, SO, D], F16, tag="h5N", name="h5N")
            am = spool.tile([P, SO], F32, tag="s4", name="am")
            for i in range(SO):
                for j in range(DO):
                    pst = psum2.tile([P, P], F16, tag="p512", name="pst")
                    nc.tensor.transpose(pst[:], h5[:, j, i * P:(i + 1) * P],
                                        cst["I128"])
                    sc.copy(h5N[:, i, j * P:(j + 1) * P], pst[:])
                v.tensor_reduce(am[:, i:i + 1], h5N[:, i, :], AX.X, OP.max,
                                apply_absolute_value=True)
            v.tensor_scalar_add(am[:], am[:], 1e-8)
            s127 = spool.tile([P, SO], F32, tag="s4", name="s127")
            v.reciprocal(s127[:], am[:])
            v.tensor_scalar_mul(s127[:], s127[:], 127.0)
            am127 = spool.tile([P, SO], F32, tag="s4", name="am127")
            v.tensor_scalar_mul(am127[:], am[:], 1.0 / 127.0)
            # quantize in N layout (scales are per-partition there), then
            # PE-transpose back to T layout -- no DRAM roundtrip
            h5qN = fp.tile([P, SO, D], F16, tag="h5qN", name="h5qN")
            for i in range(SO):
                qt = spool.tile([P, D], F32, tag="s4", name="qt")
                v.tensor_scalar_mul(qt[:], h5N[:, i, :], s127[:, i:i + 1])
                v.tensor_scalar(qt[:], qt[:], MAGIC, MAGIC, OP.add, OP.subtract)
                v.tensor_scalar_mul(h5qN[:, i, :], qt[:], am127[:, i:i + 1])
            h5q = fp.tile([P, DO, S], F16, tag="h5q", name="h5q")
            for j in range(DO):
                for i in range(SO):
                    pst = psum2.tile([P, P], F16, tag="p512", name="pst")
                    nc.tensor.transpose(pst[:], h5qN[:, i, j * P:(j + 1) * P],
                                        cst["I128"])
                    sc.copy(h5q[:, j, i * P:(i + 1) * P], pst[:])
            if DBG:
                sy.dma_start(t["d_h5q"][:], h5q[:])

            def h5q_T(ko, qh):
                lo, hi = H512[qh]
                return h5q[:, ko, lo:hi]

            # h6 = h5q + sigmoid(h5q@Wg) * (h5q@Wm)
            Wg = wring.tile([P, DO, D], F16, tag="w4", name="Wg")
            sc.dma_start(Wg[:], t["Wg"][:])
            Wm = wring.tile([P, DO, D], F16, tag="w4", name="Wm")
            sc.dma_start(Wm[:], t["Wm"][:])
            h6 = fp.tile([P, DO, S], F16, tag="h6", name="h6")
            for mo in range(DO):
                ps_g = psum2.tile([P, S], F32, tag="p1024", name="ps_g")
                ps_m = psum2.tile([P, S], F32, tag="p1024", name="ps_m")
                for lo, hi in H512:
                    for ko in range(DO):
                        nc.tensor.matmul(ps_g[:, lo:hi],
                                         Wg[:, ko, mo * P:(mo + 1) * P],
                                         h5q[:, ko, lo:hi],
                                         start=(ko == 0), stop=(ko == DO - 1))
                    for ko in range(DO):
                        nc.tensor.matmul(ps_m[:, lo:hi],
                                         Wm[:, ko, mo * P:(mo + 1) * P],
                                         h5q[:, ko, lo:hi],
                                         start=(ko == 0), stop=(ko == DO - 1))
                sg = spool.tile([P, S], F16, tag="s4", name="sg")
                sc.activation(sg[:], ps_g[:], AF.Sigmoid)
                gm = spool.tile([P, S], F16, tag="s4", name="gm")
                v.tensor_tensor(gm[:], sg[:], ps_m[:], OP.mult)
                v.tensor_tensor(h6[:, mo, :], h5q[:, mo, :], gm[:], OP.add)
            if DBG:
                sy.dma_start(t["d_h6"][:], h6[:])

            # fc + residual + LayerNorm (N layout)
            fcW = wring.tile([P, DO, D], F16, tag="w4", name="fcW")
            sc.dma_start(fcW[:], t["fcW"][:])
            outN = fp.tile([P, SO, D], F32, tag="outN", name="outN")
            for st in range(SO):
                ps = psum2.tile([P, D], F32, tag="p512", name="ps_f")
                for ko in range(DO):
                    nc.tensor.matmul(ps[:], h6[:, ko, st * P:(st + 1) * P],
                                     fcW[:, ko, :], start=(ko == 0),
                                     stop=(ko == DO - 1))
                hf = spool.tile([P, D], F32, tag="s4", name="hf")
                v.tensor_tensor(hf[:], ps[:], res[:, st, :], OP.add)
                if DBG:
                    sy.dma_start(t["d_preln"][:, st, :], hf[:])
                bst = spool.tile([P, 6], F32, tag="s4", name="bst")
                v.bn_stats(bst[:], hf[:])
                mv = spool.tile([P, 2], F32, tag="s4", name="mv")
                v.bn_aggr(mv[:], bst[:])
                nmu = spool.tile([P, 1], F32, tag="s4", name="nmu")
                v.tensor_scalar_mul(nmu[:], mv[:, 0:1], -1.0)
                sqs = spool.tile([P, 1], F32, tag="s4", name="sqs")
                v.tensor_scalar_add(sqs[:], mv[:, 1:2], 1e-6)
                rstd = spool.tile([P, 1], F32, tag="s4", name="rstd")
                sc.activation(rstd[:], sqs[:], AF.Sqrt)
                v.reciprocal(rstd[:], rstd[:])
                cen = spool.tile([P, D], F32, tag="s4", name="cen")
                v.tensor_scalar(cen[:], hf[:], nmu[:, 0:1], rstd[:, 0:1],
                                OP.add, OP.mult)
                v.tensor_tensor(cen[:], cen[:], lnp[:, 0, :], OP.mult)
                v.tensor_tensor(outN[:, st, :], cen[:], lnp[:, 1, :], OP.add)
                sy.dma_start(t["out"][:, st, :], outN[:, st, :])
    nc.compile()
    return nc


# ---------------- host side ----------------

_CACHE = {}

def _bf(x):
    return np.asarray(x).astype(np.float16)

def _f32(x):
    return np.ascontiguousarray(np.asarray(x), dtype=np.float32)

def _prep_weights(i):
    w = {}
    for m in MAMBAS:
        w[f"{m}_Wdt"] = _bf(i[f"{m}_Wdt"].reshape(DO, P, D).transpose(1, 0, 2))
        prm = np.zeros((P, 72), np.float32)
        prm[:, 0:64] = i[f"{m}_A"].reshape(DO, P, N).transpose(1, 0, 2)\
            .reshape(P, DO * N)
        prm[:, 64:68] = i[f"{m}_bdt"].reshape(DO, P).T
        prm[:, 68:72] = i[f"{m}_D"].reshape(DO, P).T
        w[f"{m}_prm"] = prm
        wbc = np.zeros((P, 2, DO, N), np.float16)
        wbc[:, 0] = _bf(i[f"{m}_WB"].reshape(DO, P, N).transpose(1, 0, 2))
        wbc[:, 1] = _bf(i[f"{m}_WC"].reshape(DO, P, N).transpose(1, 0, 2))
        w[f"{m}_wbc"] = wbc
    for name, key in (("Wq", "Wq"), ("Wk", "Wk"), ("Wv", "Wv"), ("Wo", "Wo"),
                      ("Wg", "mb_Wg"), ("Wm", "mb_Wm"), ("fcW", "fc_W")):
        w[name] = _bf(i[key].reshape(DO, P, D).transpose(1, 0, 2))
    w["bo_bc"] = np.broadcast_to(_bf(i["bo"]).reshape(1, D), (P, D)).copy()
    w["fcb"] = _bf(i["fc_b"].reshape(1, D))
    # host-side BitNet ternary quantization (weights-only computation)
    bnW = np.asarray(i["bn_W"], np.float64)                 # (L, S, S)
    s = np.abs(bnW).mean(axis=(1, 2), keepdims=True) + 1e-8
    TW = np.clip(np.round(bnW / s), -1.0, 1.0) * s          # (L, S, S)
    w["TW"] = _bf(TW.reshape(L, SO, P, S).transpose(0, 2, 1, 3))
    w["bnb"] = _bf(i["bn_b"].reshape(L, 1, S))
    w["lqW"] = _bf(i["lq_W"].reshape(L, DO, P, D).transpose(0, 2, 1, 3))
    w["lqb"] = _f32(i["lq_b"].reshape(L, DO, P).transpose(2, 1, 0))
    # host-side liquid gates: g = sigmoid(ltau); ship [g/L, (1-g)/L]
    g = 1.0 / (1.0 + np.exp(-np.asarray(i["lq_tau"], np.float64)))  # (L, D)
    gq = np.stack([g / L, (1.0 - g) / L])                   # (2, L, D)
    w["gpq"] = _f32(gq.reshape(2, L, DO, P).transpose(3, 0, 2, 1))
    w["lnsc"] = _f32(i["ln_scale"].reshape(1, D))
    w["lnbi"] = _f32(i["ln_bias"].reshape(1, D))
    cpk = np.zeros((P, 2, P), np.float16)
    cpk[:, 0] = _bf(np.eye(P))
    cpk[:, 1] = _bf(np.ones((P, P)))
    w["cpack"] = cpk
    return w


def kernel(**inputs):
    i = {k: np.asarray(v) for k, v in inputs.items()}
    B = i["x"].shape[0]
    assert B == 8 and i["x"].shape[1] == S and i["x"].shape[2] == D

    mask = i["mask"].astype(np.float32)
    use_mask = not bool(i["mask"].all())
    key = f"nc{int(use_mask)}"
    if key not in _CACHE:
        _CACHE[key] = build_program(use_mask)
    nc = _CACHE[key]

    w = _prep_weights(i)
    in_maps = []
    for b in range(B):
        xb = i["x"][b].astype(np.float32)
        m = dict(w)
        m["xT"] = _bf(xb.T.reshape(DO, P, S).transpose(1, 0, 2))
        m["xN"] = _f32(xb.reshape(SO, P, D).transpose(1, 0, 2))
        m["mask_bf"] = _bf(mask[b].reshape(1, S))
        m["mask_N"] = _f32(mask[b].reshape(SO, P).T)
        if use_mask:
            m["maskbias_k"] = _f32(((mask[b] - 1.0) * 1e9).reshape(1, S))
        in_maps.append(m)

    res = run_bass_kernel_spmd(nc, in_maps, list(range(B)))
    outs = []
    for b in range(B):
        o = np.asarray(res.results[b]["out"])
        outs.append(o.transpose(1, 0, 2).reshape(S, D))
    return np.stack(outs).astype(np.float32)


if __name__ == "__main__":
    print("building program...")
    build_program()
    print("ok")



# revision 44
# speedup vs baseline: 1.0274x; 1.0274x over previous
"""Trainium2 Bass kernel for nn_EnhancedSyntheticEmbeddingLayer.

Strategy: pure data parallelism — B=8 batch elements, one per NeuronCore.
Forward-only, so no collectives. Each core runs the full hybrid block on its
(S=1024, D=512) sequence.

Layouts: T-layout [128 part, DO=4 d-tiles, S free] (features on partitions),
N-layout [128 part, SO=8 t-tiles, D free] (tokens on partitions). All matmuls
bf16 (fp32 PE is 4x slower), fp32 PSUM accumulation.

v2 changes vs baseline (1293us -> ~1090us):
- BitNet ternary quantization + liquid gates precomputed on HOST (weights-only
  computation); the TWQ streaming pass and in-kernel quantize are gone.
- Attention: single logits pass per head in q-layout (partitions=q, free=k);
  the per-q max becomes a per-partition ACT bias and the exp emits its own
  row-sum via accum_out (no DRAM roundtrips, no second logits matmul);
  normalization folded into wq before the PE transpose to k-layout.
- Mamba: scans consolidated to NG=4 n-channels per instruction via chained
  multi-row scans with a zeroed-first-decay-column reset (memset emitted
  BEFORE the dA exps on disjoint slices so the in-order DVE queue never
  stalls); hC computed in place on hsc; hsc double-buffered so the y-reduce
  matmuls never block the next group's scan. Mamba phases run at the DVE
  floor (~13.3us per 4-channel group).
- bnb/bo/fcb biases applied on DVE instead of 1-row PE matmuls; act_quant
  runs in N-layout (per-partition scales, no DRAM roundtrip).

Known dead ends (measured): gpsimd tensor_tensor is 3.5x slower than DVE and
its scan/free-axis-reduce opcodes don't exist on Pool; dma_start_transpose
races its consumers under load (multi-core corruption) and contends for
shared DMA bandwidth; matmul free size is capped at 512 (one PSUM bank) by
the ISA even though bass compiles 1024.

v3 session findings (all measured on hw):
- Offloading dBx/u/sparse mults to gpsimd REGRESSES 240us: Pool TTs run
  ~9us/[128,4096] and the SBUF contention slows concurrent DVE scans +22%
  and TTs +85%. Do not put Pool work in the mamba windows.
- fp8e4 DoubleRow bitnet (ternary weights exact in fp8, x16-upscaled lqW,
  h2 via sc.copy) works and compresses the bitnet window ~25us, but costs
  3x l2 error (3.1e-3 -> 9.2e-3). Left implemented behind FP8_BITNET=False.
- bfloat16 instead of float16 for all PE operands: identical wall time
  (379ns sustained per 512-free matmul either way) and 3x worse l2. The
  2.4GHz/213ns "warm" rate is not reachable here: the HAM/activity throttle
  holds K=4/8 for ~775us of the kernel (throttle_activity_1 util cap 0.5).
- Micro-scheduling (weight-DMA prefetch via the idle gpsimd queue, TW
  double-buffer prefetch, q/k mo-interleave, res moved after bitnet, psum
  alternation, bnb bias on PE): each wall-neutral within +-2.5us noise.
  The wall is pinned by per-phase engine floors: 3 mamba windows ~225us
  each at the DVE floor (scan 2.08 cyc/elem = 8.7us + dBx 2.3 + hC 2.3 per
  4-channel group), attn+bitnet ~275us at the throttled PE rate, final
  ~90us. DVE total busy 913us of 1089us wall.
- The per-(d,n,t) scan work is irreducible: delta is per-d so the SSD/
  chunked-matmul reformulation needs per-partition decay operators the PE
  cannot express; log-space cumsum via PE triangular matmuls adds more
  elementwise expansion passes (stride-0 broadcasts drop DVE to 1x) than
  the scan costs. Attention logits reach +-130 scaled, so the softmax max
  subtraction cannot be dropped (fp32 exp overflows at 88).

v4 (1088 -> 1074us measured):
- Attention head-pipelining: head h+1's logits matmuls emitted before head
  h's transposes/AV, filling the ~4-5us/head PE idle at each softmax tail
  with independent work (aw bufs=2 holds both in-flight wq's; mxn/sums/rq
  moved to a dedicated 2-deep pool). -14.6us, bit-identical output. The
  winning pattern: refill the PE queue with independent matmuls across a
  latency chain. The losing pattern (iter5/iter6, +100..200us each,
  reverted): inserting ACT ops that depend on fresh PE work between
  latency-critical ACT ops -- the strict in-order ACT queue head-blocks.
- fp32-scan-accumulator probe (is the scan's 2cyc/elem fp16-feedback?)
  needs +13KB/part SBUF beyond capacity; not worth the BmF/CmF re-read
  cascade given ~10% prior (serial ALU feedback latency more likely).
- sm's B/C projection accumulated per-mo inside bitnet's last gate loop
  (post_mo hook; psum2 free after l2's bd matmuls): fills the 19us PE gap
  at bitnet's tail, pulls the staging roundtrip earlier. 1074 -> 1066.6us.
- bitnet l0's bd mo0/mo1 accumulated inside the attention Wo stage (TW0
  hoisted to a pool spanning both phases, closed before sm): net-zero
  matmul move. 1066.6 -> 1061.5us. mo2 extension via psum1: flat.
- Holding p512 psum accumulators ACROSS a phase boundary: +205us
  regression, reproduced twice (m2 delta-mo0 held through sm's dk loop;
  sm delta-mo0 held across bitnet->sm). Identical signature both times:
  scan spacing 13.3 -> 16.0us/group in ALL mambas including untouched
  ones (global semaphore/queue perturbation). p1024 cross-phase hooks
  (pre_bc_m2, post_mo_sm, bd_state) are fine; p512 holds are not.
- PE warm-up burst at t=0, retested alone after exoneration: kept
  (1060.7 -> 1060.0us; its p512 slot is read back at ~7us, not a
  cross-phase hold). Best verified: 1059995 ns, bit-identical output.
"""
import numpy as np
import ml_dtypes

import concourse.bass as bass
import concourse.mybir as mybir
import concourse.tile as tile
from concourse import bacc
from concourse.bass_utils import run_bass_kernel_spmd
from contextlib import ExitStack

F32 = mybir.dt.float32
F16 = mybir.dt.float16
F8 = mybir.dt.float8e4
AF = mybir.ActivationFunctionType
OP = mybir.AluOpType
AX = mybir.AxisListType
PM = mybir.MatmulPerfMode

FP8_BITNET = False  # bitnet GEMMs in fp8e4 DoubleRow (2 k-tiles / instr)
LQ_UPSCALE = 16.0   # lq_W ~N(0,0.02) sits in e4m3 denormal range; x16 on
                    # host, /16 via the tanh ACT scale

P = 128
S = 1024
D = 512
N = 16
L = 3
SO = 8
DO = 4
NH = 4
NG = 4   # n's per scan group
MAGIC = 1.5 * 2.0**23

DBG = False

MAMBAS = ("m1", "sm", "m2")
H512 = [(0, 512), (512, 1024)]


def _decl(nc, use_mask):
    t = {}
    def inp(name, shape, dt):
        t[name] = nc.declare_dram_parameter(name, list(shape), dt, isOutput=False)
    def outp(name, shape, dt):
        t[name] = nc.declare_dram_parameter(name, list(shape), dt, isOutput=True)

    inp("xT", (P, DO, S), F16)
    inp("xN", (P, SO, D), F32)
    inp("mask_bf", (1, S), F16)
    inp("mask_N", (P, SO), F32)
    if use_mask:
        inp("maskbias_k", (1, S), F32)
    for m in MAMBAS:
        inp(f"{m}_Wdt", (P, DO, D), F16)
        inp(f"{m}_prm", (P, 72), F32)        # [0:64]=A_sc [64:68]=bdt [68:72]=D
        inp(f"{m}_wbc", (P, 2, DO, N), F16)  # [:,0]=WB [:,1]=WC
    for w in ("Wq", "Wk", "Wv", "Wo", "Wg", "Wm", "fcW"):
        inp(w, (P, DO, D), F16)
    inp("bo_bc", (P, D), F16)
    inp("fcb", (1, D), F16)
    if FP8_BITNET:
        inp("TW", (L, P, SO, S), F8)         # host: ternary/s in {-1,0,1}
        inp("bns", (P, L), F32)              # per-layer ternary scale s
        inp("lqW", (L, P, DO, D), F8)        # host: lq_W * LQ_UPSCALE
    else:
        inp("TW", (L, P, SO, S), F16)        # host-quantized ternary weights
        inp("lqW", (L, P, DO, D), F16)
    inp("bnb", (L, 1, S), F16)
    inp("lqb", (P, DO, L), F32)
    inp("gpq", (P, 2, DO, L), F32)           # host: [g/L, (1-g)/L]
    inp("lnsc", (1, D), F32)
    inp("lnbi", (1, D), F32)
    inp("cpack", (P, 2, P), F16)             # [:,0]=I128 [:,1]=ones128
    outp("out", (P, SO, D), F32)

    t["stg_b"] = nc.dram_tensor("stg_b", [1, N * S], F16)
    t["stg_c"] = nc.dram_tensor("stg_c", [1, N * S], F16)
    t["stg_b2"] = nc.dram_tensor("stg_b2", [1, N * S], F16)
    t["stg_c2"] = nc.dram_tensor("stg_c2", [1, N * S], F16)
    t["stg_s"] = nc.dram_tensor("stg_s", [NH, S], F32)
    t["stg_s2"] = nc.dram_tensor("stg_s2", [NH, S], F32)
    t["stg_am"] = nc.dram_tensor("stg_am", [2, S], F32)
    if DBG:
        outp("d_delta1", (P, DO, S), F16)
        outp("d_u1", (P, DO, S), F16)
        outp("d_h1", (P, DO, S), F16)
        outp("d_h2", (P, SO, D), F16)
        outp("d_qT", (P, DO, S), F16)
        outp("d_kT", (P, DO, S), F16)
        outp("d_vN", (P, SO, D), F16)
        outp("d_h3", (P, DO, S), F16)
        outp("d_h4", (P, DO, S), F16)
        outp("d_h5", (P, DO, S), F16)
        outp("d_h5q", (P, DO, S), F16)
        outp("d_h6", (P, DO, S), F16)
        outp("d_preln", (P, SO, D), F32)
    return t


def build_mamba(nc, tc, t, cst, spool, wring, psum2, psum1,
                pfx, x_T, out_h, sparse, dbg=None,
                pre_bc=None, skip_bc=False, stg2=False):
    """x_T bf16 [P,DO,S] -> out_h bf16 [P,DO,S]. Opens its own phase pools."""
    v, sc, sy, gp = nc.vector, nc.scalar, nc.sync, nc.gpsimd
    with nc.named_scope(f"mamba_{pfx}"), \
         tc.tile_pool(name=f"{pfx}_p1", bufs=1) as mp1, \
         tc.tile_pool(name=f"{pfx}_pa", bufs=2) as mpa, \
         tc.tile_pool(name=f"{pfx}_ph", bufs=2) as mph, \
         tc.tile_pool(name=f"{pfx}_ps", bufs=1) as mps:
        Wdt = wring.tile([P, DO, D], F16, tag="w4", name="Wdt")
        sy.dma_start(Wdt[:], t[f"{pfx}_Wdt"][:])
        prm = mp1.tile([P, 72], F32, tag="prm", name="prm")
        sy.dma_start(prm[:], t[f"{pfx}_prm"][:])
        wbc = mp1.tile([P, 2, DO, N], F16, tag="wbc", name="wbc")
        sy.dma_start(wbc[:], t[f"{pfx}_wbc"][:])
        A_sc = prm[:, 0:64]
        bdt = prm[:, 64:68]
        D_sc = prm[:, 68:72]

        # Bm^T, Cm^T -> DRAM -> broadcast to all partitions
        stg_bc = (t["stg_b2"], t["stg_c2"]) if stg2 else \
                 (t["stg_b"], t["stg_c"])
        if not skip_bc:
            for ci, stg in enumerate(stg_bc):
                ps = psum2.tile([P, S], F32, tag="p1024", name="ps_bc")
                for lo, hi in H512:
                    for ko in range(DO):
                        nc.tensor.matmul(ps[:N, lo:hi], wbc[:, ci, ko, :],
                                         x_T[:, ko, lo:hi],
                                         start=(ko == 0), stop=(ko == DO - 1))
                bc_sb = spool.tile([N, S], F16, tag="s4", name="bc_sb")
                v.tensor_copy(bc_sb[:], ps[:N, :])
                sy.dma_start(stg.rearrange("o (n t) -> (o n) t", n=N), bc_sb[:])
        # broadcast reads split per scan group so the first group's dBx can
        # start as soon as its slice lands (shorter phase-entry latency)
        BmF = mp1.tile([P, N, S], F16, tag="BmF", name="BmF")
        CmF = mp1.tile([P, N, S], F16, tag="CmF", name="CmF")
        for g in range(N // NG):
            sl = slice(g * NG * S, (g + 1) * NG * S)
            sy.dma_start(
                BmF[:, g * NG:(g + 1) * NG, :].rearrange("p n t -> p (n t)"),
                stg_bc[0][0:1, sl].to_broadcast((P, NG * S)))
            sy.dma_start(
                CmF[:, g * NG:(g + 1) * NG, :].rearrange("p n t -> p (n t)"),
                stg_bc[1][0:1, sl].to_broadcast((P, NG * S)))

        # delta^T = softplus(Wdt^T @ x^T + bdt)  (bf16)
        # delta^T = softplus(Wdt^T @ x^T + bdt)  (bf16)
        delta = mp1.tile([P, DO, S], F16, tag="delta", name="delta")
        for mo in range(DO):
            ps = psum2.tile([P, S], F32, tag="p1024", name="ps_d")
            for lo, hi in H512:
                for ko in range(DO):
                    nc.tensor.matmul(ps[:, lo:hi],
                                     Wdt[:, ko, mo * P:(mo + 1) * P],
                                     x_T[:, ko, lo:hi],
                                     start=(ko == 0), stop=(ko == DO - 1))
            # softplus(x) = ln(1 + exp(x)); no softplus LUT in this build.
            # |x| stays O(10) here so exp can't overflow fp32.
            e1 = spool.tile([P, S], F32, tag="s4", name="e1")
            sc.activation(e1[:], ps[:], AF.Exp, bias=bdt[:, mo:mo + 1])
            sc.activation(delta[:, mo, :], e1[:], AF.Ln, bias=1.0)

        if sparse:
            # delta = where(delta > mean_d(delta), delta, 0); mean over d per t
            psm = psum2.tile([P, S], F32, tag="p1024", name="psm")
            for lo, hi in H512:
                for ko in range(DO):
                    nc.tensor.matmul(psm[:, lo:hi], cst["ones128"],
                                     delta[:, ko, lo:hi],
                                     start=(ko == 0), stop=(ko == DO - 1))
            for mo in range(DO):
                msk = spool.tile([P, S], F16, tag="s4", name="msk")
                v.scalar_tensor_tensor(msk[:], psm[:], 1.0 / D, delta[:, mo, :],
                                       OP.mult, OP.is_lt)
                v.tensor_tensor(delta[:, mo, :], delta[:, mo, :], msk[:], OP.mult)

        # u = delta * x (per d-block, so dk 0's scan chain starts as soon as
        # delta[:, 0, :] lands rather than after the whole projection)
        u = mp1.tile([P, DO, S], F16, tag="u", name="u")
        for mo in range(DO):
            v.tensor_tensor(u[:, mo, :], delta[:, mo, :], x_T[:, mo, :],
                            OP.mult)

        # scans: per dk, 2 groups of NG=8 n's. Within a group one chained
        # multi-row scan handles all 8 recurrences; dA[:,j,0]=0 for j>0 resets
        # the carried state at each row boundary (the true a_0 only multiplies
        # h_{-1}=0, so losing it is harmless).
        for dk in range(DO):
            ps_y = psum1.tile([P, S], F32, tag="ypsum", name="ps_y")
            first = True
            for g in range(N // NG):
                dA = mpa.tile([P, NG, S], F16, tag="dA", name="dA")
                # zero first decay col of rows 1..NG-1 (scan chain reset);
                # emitted BEFORE the exps (disjoint slices) so the in-order
                # DVE queue never stalls on the ACT exps here.
                v.memset(dA[:, 1:NG, 0:1], 0.0)
                for j in range(NG):
                    n = g * NG + j
                    if j == 0:
                        sc.activation(dA[:, 0, :], delta[:, dk, :], AF.Exp,
                                      scale=A_sc[:, dk * N + n: dk * N + n + 1])
                    else:
                        sc.activation(dA[:, j, 1:], delta[:, dk, 1:], AF.Exp,
                                      scale=A_sc[:, dk * N + n: dk * N + n + 1])
                dBx = mps.tile([P, NG, S], F16, tag="dBx", name="dBx")
                v.tensor_tensor(dBx[:],
                                u[:, dk, None, :].to_broadcast((P, NG, S)),
                                BmF[:, g * NG:(g + 1) * NG, :], OP.mult)
                hsc = mph.tile([P, NG, S], F16, tag="hsc", name="hsc")
                v.tensor_tensor_scan(hsc.rearrange("p g s -> p (g s)"),
                                     dA.rearrange("p g s -> p (g s)"),
                                     dBx.rearrange("p g s -> p (g s)"),
                                     0.0, OP.mult, OP.add)
                # hC in place on hsc (elementwise same-index aliasing is safe)
                v.tensor_tensor(hsc[:], hsc[:], CmF[:, g * NG:(g + 1) * NG, :],
                                OP.mult)
                for j in range(NG):
                    last = (g == N // NG - 1) and (j == NG - 1)
                    for lo, hi in H512:
                        nc.tensor.matmul(ps_y[:, lo:hi], cst["I128"],
                                         hsc[:, j, lo:hi], start=first, stop=last)
                    first = False
            v.scalar_tensor_tensor(out_h[:, dk, :], x_T[:, dk, :],
                                   D_sc[:, dk:dk + 1], ps_y[:], OP.mult, OP.add)
            if pre_bc is not None:
                pre_bc(dk)
        if dbg == 1 and DBG:
            sy.dma_start(t["d_delta1"][:], delta[:])
            sy.dma_start(t["d_u1"][:], u[:])


def build_attention(nc, tc, t, cst, spool, wring, psum2, psum1, h1, h2,
                    use_mask, post_st=None):
    """h1 bf16 [P,DO,S] -> h2 bf16 [P,SO,D].

    Single logits pass per head in q-layout (partitions=q, free=k): the per-q
    max is a per-partition ACT bias. w is then PE-transposed to k-layout for
    the sum/o matmuls. Softmax normalization is deferred past the output
    projection (a per-token ACT scale in N-layout) since diag(r)(o^T Wo) =
    (diag(r) o^T) Wo."""
    v, sc, sy, gp = nc.vector, nc.scalar, nc.sync, nc.gpsimd
    with nc.named_scope("attn"), tc.tile_pool(name="attnp", bufs=1) as ap, \
         tc.tile_pool(name="attnw", bufs=2) as aw, \
         tc.tile_pool(name="asml", bufs=2) as asml:
        # weight DMAs on the idle gpsimd queue: the scalar queue is still
        # draining m1's exp stream at this point, which delayed these ~10us
        # past window start (measured 9.8us PE gap at attn entry)
        Wq = wring.tile([P, DO, D], F16, tag="w4", name="Wq")
        gp.dma_start(Wq[:], t["Wq"][:])
        Wk = wring.tile([P, DO, D], F16, tag="w4", name="Wk")
        gp.dma_start(Wk[:], t["Wk"][:])
        Wv = wring.tile([P, DO, D], F16, tag="w4", name="Wv")
        gp.dma_start(Wv[:], t["Wv"][:])
        Wo = wring.tile([P, DO, D], F16, tag="w4", name="Wo")
        gp.dma_start(Wo[:], t["Wo"][:])
        bo_bc = spool.tile([P, D], F16, tag="s4", name="bo_bc")
        gp.dma_start(bo_bc[:], t["bo_bc"][:])

        qT = ap.tile([P, DO, S], F16, tag="qT", name="qT")
        kT = ap.tile([P, DO, S], F16, tag="kT", name="kT")
        # mo-interleaved q/k so head 0's logits start after 2 psum drains
        for mo in range(DO):
            for (W, dst) in ((Wq, qT), (Wk, kT)):
                ps = psum2.tile([P, S], F32, tag="p1024", name="ps_qk")
                for lo, hi in H512:
                    for ko in range(DO):
                        nc.tensor.matmul(ps[:, lo:hi],
                                         W[:, ko, mo * P:(mo + 1) * P],
                                         h1[:, ko, lo:hi],
                                         start=(ko == 0), stop=(ko == DO - 1))
                sc.copy(dst[:, mo, :], ps[:])
        vN = ap.tile([P, SO, D], F16, tag="vN", name="vN")
        for st in range(SO):
            ps = psum2.tile([P, D], F32, tag="p512", name="ps_v")
            for ko in range(DO):
                nc.tensor.matmul(ps[:], h1[:, ko, st * P:(st + 1) * P],
                                 Wv[:, ko, :], start=(ko == 0), stop=(ko == DO - 1))
            sc.copy(vN[:, st, :], ps[:])

        if use_mask:
            mrow = spool.tile([1, S], F32, tag="s4", name="mrow")
            sy.dma_start(mrow[:], t["maskbias_k"][:])

        scale = 1.0 / float(np.sqrt(D // NH))
        oT = ap.tile([P, DO, S], F16, tag="oT", name="oT")

        def sm_part(hd):
            # logits in q-layout; per-q max -> ACT bias for the exp; row sums
            # as q-layout free reduces (no matmul, no DRAM roundtrips);
            # normalization folded into wq before the transpose.
            wq = aw.tile([P, SO, S], F16, tag="wq", name="wq")
            mxn = asml.tile([P, SO], F32, tag="mxn", name="mxn")
            sums = asml.tile([P, SO], F32, tag="sums", name="sums")
            for qt in range(SO):
                # rotate logits through 3 PSUM slots (2x p1024 + ypsum) so
                # the max/exp latency pipeline doesn't stall the PE
                if qt % 3 == 2:
                    ps_l = psum1.tile([P, S], F32, tag="ypsum", name="ps_l")
                else:
                    ps_l = psum2.tile([P, S], F32, tag="p1024", name="ps_l")
                for lo, hi in H512:
                    nc.tensor.matmul(ps_l[:, lo:hi],
                                     qT[:, hd, qt * P:(qt + 1) * P],
                                     kT[:, hd, lo:hi], start=True,
                                     stop=(not use_mask))
                    if use_mask:
                        # masked keys: +(-1e9) -> exp()=0 (scaled by `scale`,
                        # still ~-1e8, far below fp32 exp underflow)
                        nc.tensor.matmul(ps_l[:, lo:hi], cst["ones1"],
                                         mrow[0:1, lo:hi], start=False,
                                         stop=True)
                mx = spool.tile([P, 1], F32, tag="s4", name="mx")
                v.tensor_reduce(mx[:], ps_l[:], AX.X, OP.max)
                v.tensor_scalar(mxn[:, qt:qt + 1], mx[:], -scale, 0.0,
                                OP.mult, OP.add)
                # the exp emits its own row sum via the ACT accumulator
                sc.activation(wq[:, qt, :], ps_l[:], AF.Exp, scale=scale,
                              bias=mxn[:, qt:qt + 1],
                              accum_out=sums[:, qt:qt + 1])
            r_q = asml.tile([P, SO], F32, tag="rq", name="r_q")
            v.reciprocal(r_q[:], sums[:])
            return wq, r_q

        def av_part(hd, wq, r_q):
            for qt in range(SO):
                v.tensor_scalar_mul(wq[:, qt, :], wq[:, qt, :],
                                    r_q[:, qt:qt + 1])
            # PE-transpose normalized w to k-layout [k, q]
            wT = aw.tile([P, SO, S], F16, tag="wT", name="wT")
            for kt in range(SO):
                pst = psum2.tile([P, SO, P], F16, tag="p512", name="pst")
                for qt in range(SO):
                    nc.tensor.transpose(pst[:, qt, :],
                                        wq[:, qt, kt * P:(kt + 1) * P],
                                        cst["I128"])
                sc.copy(wT[:, kt, :], pst.rearrange("p a b -> p (a b)"))
            # o = w^T v per q-half (1-bank PSUM tiles)
            for qh, (lo, hi) in enumerate(H512):
                ps_o = psum2.tile([P, 512], F32, tag="p512", name="ps_o")
                for kt in range(SO):
                    nc.tensor.matmul(ps_o[:],
                                     vN[:, kt, hd * P:(hd + 1) * P],
                                     wT[:, kt, lo:hi],
                                     start=(kt == 0), stop=(kt == SO - 1))
                sc.copy(oT[:, hd, lo:hi], ps_o[:])

        # head-pipelined: head h+1's logits matmuls are emitted before head
        # h's transposes/AV, so the PE fills head h's softmax-latency tail
        # (~4-5us/head measured idle) with independent work. aw bufs=2 gives
        # the two in-flight heads separate wq slots.
        pend = None
        for hd in range(NH):
            cur = sm_part(hd)
            if pend is not None:
                av_part(*pend)
            pend = (hd, *cur)
        av_part(*pend)

        if DBG:
            sy.dma_start(t["d_qT"][:], qT[:])
            sy.dma_start(t["d_kT"][:], kT[:])
            sy.dma_start(t["d_vN"][:], vN[:])
        for st in range(SO):
            ps = psum2.tile([P, D], F32, tag="p512", name="ps_a")
            for ko in range(DO):
                nc.tensor.matmul(ps[:], oT[:, ko, st * P:(st + 1) * P],
                                 Wo[:, ko, :], start=(ko == 0),
                                 stop=(ko == DO - 1))
            v.tensor_tensor(h2[:, st, :], ps[:], bo_bc[:], OP.add)
            if post_st is not None:
                post_st(st)


def build_bitnet(nc, tc, t, cst, spool, wring, psum2, psum1, h2, h3,
                 post_mo=None, tw0=None, bd_pre=None):
    """L x (ternary seq-dense + liquid gate), mean over L.
    h2 bf16 [P,SO,D] -> h3 bf16 [P,DO,S]. TW host-precomputed."""
    v, sc, sy, gp = nc.vector, nc.scalar, nc.sync, nc.gpsimd
    WDT = F8 if FP8_BITNET else F16
    with nc.named_scope("bitnet"), \
         tc.tile_pool(name="bitp1", bufs=1) as bp1, \
         tc.tile_pool(name="bitp2", bufs=2) as bp2:
        lqW = [wring.tile([P, DO, D], WDT, tag="w4", name=f"lqW{l}")
               for l in range(L)]
        for l in range(L):
            gp.dma_start(lqW[l][:], t["lqW"][l])
        # bnb broadcast to all partitions; added on DVE (frees PE matmuls)
        bnb = bp1.tile([P, L, S], F16, tag="bnb", name="bnb")
        for l in range(L):
            sy.dma_start(bnb[:, l, :], t["bnb"][l].to_broadcast((P, S)))
        bprm = bp1.tile([P, DO, L], F32, tag="bprm", name="bprm")
        sy.dma_start(bprm[:], t["lqb"][:])
        gpq = bp1.tile([P, 2, DO, L], F32, tag="gpq", name="gpq")
        sy.dma_start(gpq[:], t["gpq"][:])
        if FP8_BITNET:
            bns = bp1.tile([P, L], F32, tag="bns", name="bns")
            sy.dma_start(bns[:], t["bns"][:])
            # h2 copy in fp8 for the DoubleRow stationary
            h2q = bp1.tile([P, SO, D], F8, tag="h2q", name="h2q")
            for st in range(SO):
                sc.copy(h2q[:, st, :], h2[:, st, :])
            bdT8 = bp1.tile([P, DO, S], F8, tag="bdT8", name="bdT8")

        h3f = bp1.tile([P, DO, S], F16, tag="h3f", name="h3f")
        bdT = bp1.tile([P, DO, S], F16, tag="bdT", name="bdT")
        tanh_sc = (1.0 / LQ_UPSCALE) if FP8_BITNET else 1.0
        # TW double-buffered with a 1-layer DMA prefetch (bp2 bufs=2): layer
        # l+1's 1MB load runs under layer l's matmuls instead of gating them.
        # Layer 0's TW arrives preloaded (tw0) when its first two bd mo's
        # were accumulated during the attention Wo stage.
        if tw0 is not None:
            TW_next = tw0
        else:
            TW_next = bp2.tile([P, SO, S], WDT, tag="TWs", name="TW")
            gp.dma_start(TW_next[:], t["TW"][0])
        for l in range(L):
            TW = TW_next
            if l + 1 < L:
                TW_next = bp2.tile([P, SO, S], WDT, tag="TWs", name="TW")
                gp.dma_start(TW_next[:], t["TW"][l + 1])
            for mo in range(DO):
                if l == 0 and mo < 2 and bd_pre is not None and not FP8_BITNET:
                    # accumulated during the Wo stage (h2 t-tiles land per
                    # st there): just drain
                    ps_bd = bd_pre()[mo]
                    v.tensor_tensor(bdT[:, mo, :], ps_bd[:], bnb[:, l, :],
                                    OP.add)
                    continue
                ps_bd = psum2.tile([P, S], F32, tag="p1024", name="ps_bd")
                if FP8_BITNET:
                    # fp8e4 DoubleRow: 2 k-tiles per matmul (stationary
                    # [128, 2, 128] = paired h2 t-tiles, moving [128, 2, 512])
                    for lo, hi in H512:
                        for ko in range(0, SO, 2):
                            nc.tensor.matmul(
                                ps_bd[:, lo:hi],
                                h2q[:, ko:ko + 2, mo * P:(mo + 1) * P],
                                TW[:, ko:ko + 2, lo:hi],
                                start=(ko == 0), stop=(ko == SO - 2),
                                perf_mode=PM.DoubleRow)
                    # bd = raw*s + bnb  (s = host ternary scale, folded out of
                    # TW so the fp8 weights are exact {-1,0,1})
                    v.scalar_tensor_tensor(bdT[:, mo, :], ps_bd[:],
                                           bns[:, l:l + 1], bnb[:, l, :],
                                           OP.mult, OP.add)
                    sc.copy(bdT8[:, mo, :], bdT[:, mo, :])
                else:
                    for lo, hi in H512:
                        for ko in range(SO):
                            nc.tensor.matmul(ps_bd[:, lo:hi],
                                             h2[:, ko, mo * P:(mo + 1) * P],
                                             TW[:, ko, lo:hi],
                                             start=(ko == 0), stop=(ko == SO - 1))
                    v.tensor_tensor(bdT[:, mo, :], ps_bd[:], bnb[:, l, :], OP.add)
            for mo in range(DO):
                ps_tg = psum1.tile([P, S], F32, tag="ypsum", name="ps_tg")
                if FP8_BITNET:
                    for lo, hi in H512:
                        for ko in range(0, DO, 2):
                            nc.tensor.matmul(
                                ps_tg[:, lo:hi],
                                lqW[l][:, ko:ko + 2, mo * P:(mo + 1) * P],
                                bdT8[:, ko:ko + 2, lo:hi],
                                start=(ko == 0), stop=(ko == DO - 2),
                                perf_mode=PM.DoubleRow)
                else:
                    for lo, hi in H512:
                        for ko in range(DO):
                            nc.tensor.matmul(ps_tg[:, lo:hi],
                                             lqW[l][:, ko, mo * P:(mo + 1) * P],
                                             bdT[:, ko, lo:hi],
                                             start=(ko == 0), stop=(ko == DO - 1))
                th = spool.tile([P, S], F16, tag="s4", name="th")
                sc.activation(th[:], ps_tg[:], AF.Tanh, bias=bprm[:, mo, l:l + 1],
                              scale=tanh_sc)
                if l == 0:
                    v.tensor_scalar_mul(h3f[:, mo, :], bdT[:, mo, :],
                                        gpq[:, 0, mo, l:l + 1])
                else:
                    v.scalar_tensor_tensor(h3f[:, mo, :], bdT[:, mo, :],
                                           gpq[:, 0, mo, l:l + 1], h3f[:, mo, :],
                                           OP.mult, OP.add)
                # last layer's final accumulate writes h3 directly (the
                # full-tile copy was 2.1us of serial DVE at the sm boundary)
                gate_out = h3[:, mo, :] if l == L - 1 else h3f[:, mo, :]
                v.scalar_tensor_tensor(gate_out, th[:],
                                       gpq[:, 1, mo, l:l + 1], h3f[:, mo, :],
                                       OP.mult, OP.add)
                if l == L - 1 and post_mo is not None:
                    post_mo(mo)


def build_program(use_mask=False):
    nc = bacc.Bacc("TRN2")
    t = _decl(nc, use_mask)
    with tile.TileContext(nc) as tc, ExitStack() as ctx:
        gpool = ctx.enter_context(tc.tile_pool(name="gpool", bufs=1))
        wring = ctx.enter_context(tc.tile_pool(name="wring", bufs=4))
        spool = ctx.enter_context(tc.tile_pool(name="spool", bufs=7))
        psum2 = ctx.enter_context(tc.tile_pool(name="psum2", bufs=2, space="PSUM"))
        psum1 = ctx.enter_context(tc.tile_pool(name="psum1", bufs=1, space="PSUM"))
        v, sc, sy = nc.vector, nc.scalar, nc.sync

        cpk = gpool.tile([P, 2, P], F16, tag="cpack", name="cpk")
        sy.dma_start(cpk[:], t["cpack"][:])
        cst = {"I128": cpk[:, 0, :], "ones128": cpk[:, 1, :],
               "ones1": cpk[0:1, 1, :], "cpk": cpk}
        # PE warm-up: ~7us of back-to-back transposes on cpack (arrives ~1us)
        # ramp the HAM/p-state before m1's entry matmuls land (those run at
        # 630-787ns cold vs ~430 warm). The p512 psum slot is read back and
        # released at ~7us, well before attention's first p512 use -- NOT a
        # cross-phase hold (the twice-measured +205us cadence-break trigger).
        with tc.tile_pool(name="warmp", bufs=1) as wp:
            wps = psum2.tile([P, SO, P], F16, tag="p512", name="warm_ps")
            for i in range(32):
                nc.tensor.transpose(wps[:, i % SO, :], cpk[:, 0, :],
                                    cpk[:, 0, :])
            wsink = wp.tile([P, P], F16, tag="wsink", name="wsink")
            sc.copy(wsink[:], wps[:, 0, :])
        res = gpool.tile([P, SO, D], F16, tag="res", name="res")
        xmT = gpool.tile([P, DO, S], F16, tag="hB", name="xmT")
        h2N = gpool.tile([P, SO, D], F16, tag="h2N", name="h2N")

        with tc.tile_pool(name="initp", bufs=1) as ip:
            if use_mask:
                xTr = ip.tile([P, DO, S], F16, tag="xTr", name="xTr")
                sy.dma_start(xTr[:], t["xT"][:])
                maskb = ip.tile([P, S], F16, tag="maskb", name="maskb")
                sy.dma_start(maskb[:], t["mask_bf"][0:1, :].to_broadcast((P, S)))
                v.tensor_tensor(xmT[:], xTr[:],
                                maskb[:, None, :].to_broadcast((P, DO, S)),
                                OP.mult)
            else:
                # mask is all-ones: x*mask == x; xT per d-block so m1's
                # ko=0 matmuls fire after 0.5MB instead of the full 2MB
                for j in range(DO):
                    sy.dma_start(xmT[:, j, :], t["xT"][:, j, :])

        h1 = gpool.tile([P, DO, S], F16, tag="hA", name="h1")
        build_mamba(nc, tc, t, cst, spool, wring, psum2, psum1,
                    "m1", xmT, h1, sparse=False, dbg=1)

        if DBG:
            sy.dma_start(t["d_h1"][:], h1[:])

        # TW0 spans attention+bitnet so bitnet l0's first two bd mo's can
        # accumulate inside the Wo stage (h2 t-tiles land there one per st)
        tw0_cm = tc.tile_pool(name="tw0p", bufs=1)
        tw0p = tw0_cm.__enter__()
        TW0 = tw0p.tile([P, SO, S], F16, tag="TW0", name="TW0")
        nc.gpsimd.dma_start(TW0[:], t["TW"][0])
        bd_state = {}

        def post_st_bd(st):
            if st == 0:
                bd_state["p"] = [psum2.tile([P, S], F32, tag="p1024",
                                            name=f"ps_bd0{m}")
                                 for m in range(2)]
            for m in range(2):
                ps = bd_state["p"][m]
                for lo, hi in H512:
                    nc.tensor.matmul(ps[:, lo:hi],
                                     h2N[:, st, m * P:(m + 1) * P],
                                     TW0[:, st, lo:hi],
                                     start=(st == 0), stop=(st == SO - 1))

        build_attention(nc, tc, t, cst, spool, wring, psum2, psum1, h1, h2N,
                        use_mask, post_st=post_st_bd)
        if DBG:
            sy.dma_start(t["d_h2"][:], h2N[:])

        h3 = gpool.tile([P, DO, S], F16, tag="hB", name="h3")
        # sm's B/C projection accumulated inside bitnet's last gate loop:
        # h3[:, mo, :] lands per mo there, and psum2 is free once l2's bd
        # matmuls finish -- fills the measured 19us PE gap at bitnet's tail
        # and pulls sm's staging roundtrip ~10us earlier.
        wbc_s = gpool.tile([P, 2, DO, N], F16, tag="wbcs", name="wbc_s")
        sy.dma_start(wbc_s[:], t["sm_wbc"][:])
        bcs_state = {}

        def post_mo_sm(mo):
            if mo == 0:
                bcs_state["pb"] = psum2.tile([P, S], F32, tag="p1024",
                                             name="ps_bs")
                bcs_state["pc"] = psum2.tile([P, S], F32, tag="p1024",
                                             name="ps_cs")
            for ci, key in ((0, "pb"), (1, "pc")):
                ps = bcs_state[key]
                for lo, hi in H512:
                    nc.tensor.matmul(ps[:N, lo:hi], wbc_s[:, ci, mo, :],
                                     h3[:, mo, lo:hi],
                                     start=(mo == 0), stop=(mo == DO - 1))
            if mo == DO - 1:
                for key, stg in (("pb", t["stg_b"]), ("pc", t["stg_c"])):
                    bc_sb = spool.tile([N, S], F16, tag="s4", name="bc_sbs")
                    v.tensor_copy(bc_sb[:], bcs_state[key][:N, :])
                    sy.dma_start(stg.rearrange("o (n t) -> (o n) t", n=N),
                                 bc_sb[:])

        build_bitnet(nc, tc, t, cst, spool, wring, psum2, psum1, h2N, h3,
                     post_mo=post_mo_sm, tw0=TW0,
                     bd_pre=lambda: bd_state["p"])
        tw0_cm.__exit__(None, None, None)   # free TW0's 16KB before sm opens
        # residual (x + fc_b) in N layout. Emitted here (after bitnet, where
        # DVE has slack) — when it sat between m1 and attention its 4.4us
        # fp32 TT + the 2MB xN DMA head-blocked the attention entry pipeline
        # (measured ~10us PE gap at the qk stage).
        with tc.tile_pool(name="resp", bufs=1) as rp:
            xNr = rp.tile([P, SO, D], F32, tag="xNr", name="xNr")
            sy.dma_start(xNr[:], t["xN"][:])
            fcb_bc = spool.tile([P, D], F16, tag="s4", name="fcb_bc")
            sy.dma_start(fcb_bc[:], t["fcb"][0:1, :].to_broadcast((P, D)))
            if use_mask:
                maskN = spool.tile([P, SO], F32, tag="s4", name="maskN")
                sy.dma_start(maskN[:], t["mask_N"][:])
                for st in range(SO):
                    v.tensor_scalar_mul(res[:, st, :], xNr[:, st, :],
                                        maskN[:, st:st + 1])
                for st in range(SO):
                    v.tensor_tensor(res[:, st, :], res[:, st, :], fcb_bc[:],
                                    OP.add)
            else:
                v.tensor_tensor(res[:], xNr[:],
                                fcb_bc[:, None, :].to_broadcast((P, SO, D)),
                                OP.add)
        if DBG:
            sy.dma_start(t["d_h3"][:], h3[:])

        h4 = gpool.tile([P, DO, S], F16, tag="hA", name="h4")
        # m2's B/C projections are emitted inside sm's dk loop (engines run
        # their queues in order, so only emission-interleaved work can
        # execute during sm's scans). psum2 is idle during sm's scan phase,
        # so the two accumulators can hold both ring slots across the dks.
        wbc2 = gpool.tile([P, 2, DO, N], F16, tag="wbc2", name="wbc2")
        sy.dma_start(wbc2[:], t["m2_wbc"][:])
        bc2_state = {}

        def pre_bc_m2(dk):
            if dk == 0:
                bc2_state["pb"] = psum2.tile([P, S], F32, tag="p1024",
                                             name="ps_b2")
                bc2_state["pc"] = psum2.tile([P, S], F32, tag="p1024",
                                             name="ps_c2")
            for ci, key in ((0, "pb"), (1, "pc")):
                ps = bc2_state[key]
                for lo, hi in H512:
                    nc.tensor.matmul(ps[:N, lo:hi], wbc2[:, ci, dk, :],
                                     h4[:, dk, lo:hi],
                                     start=(dk == 0), stop=(dk == DO - 1))
            if dk == DO - 1:
                for key, stg in (("pb", t["stg_b2"]), ("pc", t["stg_c2"])):
                    bc_sb = spool.tile([N, S], F16, tag="s4", name="bc_sb2")
                    v.tensor_copy(bc_sb[:], bc2_state[key][:N, :])
                    sy.dma_start(stg.rearrange("o (n t) -> (o n) t", n=N),
                                 bc_sb[:])

        build_mamba(nc, tc, t, cst, spool, wring, psum2, psum1,
                    "sm", h3, h4, sparse=True, pre_bc=pre_bc_m2, skip_bc=True)
        if DBG:
            sy.dma_start(t["d_h4"][:], h4[:])
        h5 = gpool.tile([P, DO, S], F16, tag="hB", name="h5")
        build_mamba(nc, tc, t, cst, spool, wring, psum2, psum1,
                    "m2", h4, h5, sparse=False, skip_bc=True, stg2=True)
        if DBG:
            sy.dma_start(t["d_h5"][:], h5[:])

        with nc.named_scope("final"), tc.tile_pool(name="finp", bufs=1) as fp:
            lnp = fp.tile([P, 2, D], F32, tag="lnpack", name="lnp")
            sy.dma_start(lnp[:, 0, :], t["lnsc"][0:1, :].to_broadcast((P, D)))
            sy.dma_start(lnp[:, 1, :], t["lnbi"][0:1, :].to_broadcast((P, D)))
            # act_quant: PE-transpose to N layout for per-token absmax over d
            h5N = fp.tile([P, SO, D], F16, tag="h5N", name="h5N")
            am = spool.tile([P, SO], F32, tag="s4", name="am")
            for i in range(SO):
                for j in range(DO):
                    pst = psum2.tile([P, P], F16, tag="p512", name="pst")
                    nc.tensor.transpose(pst[:], h5[:, j, i * P:(i + 1) * P],
                                        cst["I128"])
                    sc.copy(h5N[:, i, j * P:(j + 1) * P], pst[:])
                v.tensor_reduce(am[:, i:i + 1], h5N[:, i, :], AX.X, OP.max,
                                apply_absolute_value=True)
            v.tensor_scalar_add(am[:], am[:], 1e-8)
            s127 = spool.tile([P, SO], F32, tag="s4", name="s127")
            v.reciprocal(s127[:], am[:])
            v.tensor_scalar_mul(s127[:], s127[:], 127.0)
            am127 = spool.tile([P, SO], F32, tag="s4", name="am127")
            v.tensor_scalar_mul(am127[:], am[:], 1.0 / 127.0)
            # quantize in N layout (scales are per-partition there), then
            # PE-transpose back to T layout -- no DRAM roundtrip
            h5qN = fp.tile([P, SO, D], F16, tag="h5qN", name="h5qN")
            for i in range(SO):
                qt = spool.tile([P, D], F32, tag="s4", name="qt")
                v.tensor_scalar_mul(qt[:], h5N[:, i, :], s127[:, i:i + 1])
                v.tensor_scalar(qt[:], qt[:], MAGIC, MAGIC, OP.add, OP.subtract)
                v.tensor_scalar_mul(h5qN[:, i, :], qt[:], am127[:, i:i + 1])
            h5q = fp.tile([P, DO, S], F16, tag="h5q", name="h5q")
            for j in range(DO):
                for i in range(SO):
                    pst = psum2.tile([P, P], F16, tag="p512", name="pst")
                    nc.tensor.transpose(pst[:], h5qN[:, i, j * P:(j + 1) * P],
                                        cst["I128"])
                    sc.copy(h5q[:, j, i * P:(i + 1) * P], pst[:])
            if DBG:
                sy.dma_start(t["d_h5q"][:], h5q[:])

            def h5q_T(ko, qh):
                lo, hi = H512[qh]
                return h5q[:, ko, lo:hi]

            # h6 = h5q + sigmoid(h5q@Wg) * (h5q@Wm)
            Wg = wring.tile([P, DO, D], F16, tag="w4", name="Wg")
            nc.gpsimd.dma_start(Wg[:], t["Wg"][:])
            Wm = wring.tile([P, DO, D], F16, tag="w4", name="Wm")
            nc.gpsimd.dma_start(Wm[:], t["Wm"][:])
            h6 = fp.tile([P, DO, S], F16, tag="h6", name="h6")
            for mo in range(DO):
                ps_g = psum2.tile([P, S], F32, tag="p1024", name="ps_g")
                ps_m = psum2.tile([P, S], F32, tag="p1024", name="ps_m")
                for lo, hi in H512:
                    for ko in range(DO):
                        nc.tensor.matmul(ps_g[:, lo:hi],
                                         Wg[:, ko, mo * P:(mo + 1) * P],
                                         h5q[:, ko, lo:hi],
                                         start=(ko == 0), stop=(ko == DO - 1))
                    for ko in range(DO):
                        nc.tensor.matmul(ps_m[:, lo:hi],
                                         Wm[:, ko, mo * P:(mo + 1) * P],
                                         h5q[:, ko, lo:hi],
                                         start=(ko == 0), stop=(ko == DO - 1))
                sg = spool.tile([P, S], F16, tag="s4", name="sg")
                sc.activation(sg[:], ps_g[:], AF.Sigmoid)
                gm = spool.tile([P, S], F16, tag="s4", name="gm")
                v.tensor_tensor(gm[:], sg[:], ps_m[:], OP.mult)
                v.tensor_tensor(h6[:, mo, :], h5q[:, mo, :], gm[:], OP.add)
            if DBG:
                sy.dma_start(t["d_h6"][:], h6[:])

            # fc + residual + LayerNorm (N layout)
            fcW = wring.tile([P, DO, D], F16, tag="w4", name="fcW")
            nc.gpsimd.dma_start(fcW[:], t["fcW"][:])
            outN = fp.tile([P, SO, D], F32, tag="outN", name="outN")
            for st in range(SO):
                ps = psum2.tile([P, D], F32, tag="p512", name="ps_f")
                for ko in range(DO):
                    nc.tensor.matmul(ps[:], h6[:, ko, st * P:(st + 1) * P],
                                     fcW[:, ko, :], start=(ko == 0),
                                     stop=(ko == DO - 1))
                hf = spool.tile([P, D], F32, tag="s4", name="hf")
                v.tensor_tensor(hf[:], ps[:], res[:, st, :], OP.add)
                if DBG:
                    sy.dma_start(t["d_preln"][:, st, :], hf[:])
                bst = spool.tile([P, 6], F32, tag="s4", name="bst")
                v.bn_stats(bst[:], hf[:])
                mv = spool.tile([P, 2], F32, tag="s4", name="mv")
                v.bn_aggr(mv[:], bst[:])
                nmu = spool.tile([P, 1], F32, tag="s4", name="nmu")
                v.tensor_scalar_mul(nmu[:], mv[:, 0:1], -1.0)
                sqs = spool.tile([P, 1], F32, tag="s4", name="sqs")
                v.tensor_scalar_add(sqs[:], mv[:, 1:2], 1e-6)
                rstd = spool.tile([P, 1], F32, tag="s4", name="rstd")
                sc.activation(rstd[:], sqs[:], AF.Sqrt)
                v.reciprocal(rstd[:], rstd[:])
                cen = spool.tile([P, D], F32, tag="s4", name="cen")
                v.tensor_scalar(cen[:], hf[:], nmu[:, 0:1], rstd[:, 0:1],
                                OP.add, OP.mult)
                v.tensor_tensor(cen[:], cen[:], lnp[:, 0, :], OP.mult)
                v.tensor_tensor(outN[:, st, :], cen[:], lnp[:, 1, :], OP.add)
                sy.dma_start(t["out"][:, st, :], outN[:, st, :])
    nc.compile()
    return nc


# ---------------- host side ----------------

_CACHE = {}

def _bf(x):
    return np.asarray(x).astype(np.float16)

def _f32(x):
    return np.ascontiguousarray(np.asarray(x), dtype=np.float32)

def _prep_weights(i):
    w = {}
    for m in MAMBAS:
        w[f"{m}_Wdt"] = _bf(i[f"{m}_Wdt"].reshape(DO, P, D).transpose(1, 0, 2))
        prm = np.zeros((P, 72), np.float32)
        prm[:, 0:64] = i[f"{m}_A"].reshape(DO, P, N).transpose(1, 0, 2)\
            .reshape(P, DO * N)
        prm[:, 64:68] = i[f"{m}_bdt"].reshape(DO, P).T
        prm[:, 68:72] = i[f"{m}_D"].reshape(DO, P).T
        w[f"{m}_prm"] = prm
        wbc = np.zeros((P, 2, DO, N), np.float16)
        wbc[:, 0] = _bf(i[f"{m}_WB"].reshape(DO, P, N).transpose(1, 0, 2))
        wbc[:, 1] = _bf(i[f"{m}_WC"].reshape(DO, P, N).transpose(1, 0, 2))
        w[f"{m}_wbc"] = wbc
    for name, key in (("Wq", "Wq"), ("Wk", "Wk"), ("Wv", "Wv"), ("Wo", "Wo"),
                      ("Wg", "mb_Wg"), ("Wm", "mb_Wm"), ("fcW", "fc_W")):
        w[name] = _bf(i[key].reshape(DO, P, D).transpose(1, 0, 2))
    w["bo_bc"] = np.broadcast_to(_bf(i["bo"]).reshape(1, D), (P, D)).copy()
    w["fcb"] = _bf(i["fc_b"].reshape(1, D))
    # host-side BitNet ternary quantization (weights-only computation)
    bnW = np.asarray(i["bn_W"], np.float64)                 # (L, S, S)
    s = np.abs(bnW).mean(axis=(1, 2), keepdims=True) + 1e-8
    TWq = np.clip(np.round(bnW / s), -1.0, 1.0)             # (L, S, S) in {-1,0,1}
    if FP8_BITNET:
        w["TW"] = (TWq.reshape(L, SO, P, S).transpose(0, 2, 1, 3)
                   .astype(ml_dtypes.float8_e4m3fn))
        w["bns"] = _f32(np.broadcast_to(s.reshape(1, L), (P, L)))
        w["lqW"] = ((np.asarray(i["lq_W"], np.float64) * LQ_UPSCALE)
                    .reshape(L, DO, P, D).transpose(0, 2, 1, 3)
                    .astype(ml_dtypes.float8_e4m3fn))
    else:
        w["TW"] = _bf((TWq * s).reshape(L, SO, P, S).transpose(0, 2, 1, 3))
        w["lqW"] = _bf(i["lq_W"].reshape(L, DO, P, D).transpose(0, 2, 1, 3))
    w["bnb"] = _bf(i["bn_b"].reshape(L, 1, S))
    w["lqb"] = _f32(i["lq_b"].reshape(L, DO, P).transpose(2, 1, 0))
    # host-side liquid gates: g = sigmoid(ltau); ship [g/L, (1-g)/L]
    g = 1.0 / (1.0 + np.exp(-np.asarray(i["lq_tau"], np.float64)))  # (L, D)
    gq = np.stack([g / L, (1.0 - g) / L])                   # (2, L, D)
    w["gpq"] = _f32(gq.reshape(2, L, DO, P).transpose(3, 0, 2, 1))
    w["lnsc"] = _f32(i["ln_scale"].reshape(1, D))
    w["lnbi"] = _f32(i["ln_bias"].reshape(1, D))
    cpk = np.zeros((P, 2, P), np.float16)
    cpk[:, 0] = _bf(np.eye(P))
    cpk[:, 1] = _bf(np.ones((P, P)))
    w["cpack"] = cpk
    return w


def kernel(**inputs):
    i = {k: np.asarray(v) for k, v in inputs.items()}
    B = i["x"].shape[0]
    assert B == 8 and i["x"].shape[1] == S and i["x"].shape[2] == D

    mask = i["mask"].astype(np.float32)
    use_mask = not bool(i["mask"].all())
    key = f"nc{int(use_mask)}"
    if key not in _CACHE:
        _CACHE[key] = build_program(use_mask)
    nc = _CACHE[key]

    w = _prep_weights(i)
    in_maps = []
    for b in range(B):
        xb = i["x"][b].astype(np.float32)
        m = dict(w)
        m["xT"] = _bf(xb.T.reshape(DO, P, S).transpose(1, 0, 2))
        m["xN"] = _f32(xb.reshape(SO, P, D).transpose(1, 0, 2))
        m["mask_bf"] = _bf(mask[b].reshape(1, S))
        m["mask_N"] = _f32(mask[b].reshape(SO, P).T)
        if use_mask:
            m["maskbias_k"] = _f32(((mask[b] - 1.0) * 1e9).reshape(1, S))
        in_maps.append(m)

    res = run_bass_kernel_spmd(nc, in_maps, list(range(B)))
    outs = []
    for b in range(B):
        o = np.asarray(res.results[b]["out"])
        outs.append(o.transpose(1, 0, 2).reshape(S, D))
    return np.stack(outs).astype(np.float32)


if __name__ == "__main__":
    print("building program...")
    build_program()
    print("ok")

